# revision 16
# baseline (speedup 1.0000x reference)
"""Bass/Tile Trainium2 kernel for additive (Bahdanau/'cat') attention.

Problem (per batch b):
  A[i,d]      = sum_a context[i,a] * attn_w[a,d] + attn_b[d]
  O[o,d]      = sum_e output[o,e]  * dec_w[e,d]  + dec_b[d]
  scores[o,i] = sum_d query_w[d] * tanh(A[i,d] + O[o,d])   (+query_b: softmax-invariant)
  attn        = softmax_i(scores)
  mix[o,a]    = sum_i attn[o,i] * context[i,a]
  out[o,d]    = tanh([mix | output] @ out_w + out_b)

Sharding: pure data-parallel over batch, B=8 -> one batch per NeuronCore,
weights broadcast, no collectives.

Per-core structure:
  * A^T [d,i] and O^T [d,o] kept with d on partitions so the broadcast add
    A^T + O^T[:,o] is a DVE tensor_scalar (per-partition scalar), in bf16.
  * tanh batched 8 o's per ACT instruction (free dim 4096); d-chunk-outer
    so the PE gets matmul work after every ACT chunk (keeps HAM warm).
  * q-reduction over d on the PE with zero-padded stationary operand:
    lhsT QZ[:,dc,j] is [128,8] holding query_w in column j -> all 32
    matmuls of a group accumulate into ONE [8,512] PSUM bank; one cheap
    8-row DVE copy + SBUF->SBUF DMA scatters rows into scores.
  * softmax/mix/out epilogue computed in row-halves (0:32 during groups
    4..7, 32:64 at the end) to shorten the serial tail.
"""

import numpy as np

import concourse.bass as bass
import concourse.tile as tile
import concourse.bass_utils as bass_utils
from concourse import bacc, mybir
from concourse.masks import make_identity

B, OUT_LEN, IN_LEN, DEC, ATTN = 8, 64, 512, 512, 512
P = 128
F32 = mybir.dt.float32
BF16 = mybir.dt.bfloat16
AF = mybir.ActivationFunctionType

G = 8                     # o's per tanh group
NG = OUT_LEN // G         # 8 groups
DC = DEC // P             # 4 d-chunks
AC = ATTN // P            # 4 a-chunks
IC = IN_LEN // P          # 4 i-chunks
EC = DEC // P             # 4 e-chunks (decoder feature)
CC = (ATTN + DEC) // P    # 8 combined chunks
H = OUT_LEN // 2          # row half

N_CORES = 8


def _epilogue_softmax_mix(nc, h, ident_bf, scores_sb, exp_sb, sums, recip,
                          attn_sb, attn_bf, attnT_bf, ctx_bf, combT_bf, psum,
                          attn_d):
    """softmax + attn^T + mix for rows h*32..h*32+31 (all-bf16 matmuls)."""
    r0 = h * H
    sl = slice(r0, r0 + H)
    nc.scalar.activation(exp_sb[sl, :], scores_sb[sl, :], AF.Exp, accum_out=sums[sl, :])
    nc.vector.reciprocal(recip[sl, :], sums[sl, :])
    nc.vector.tensor_scalar_mul(attn_sb[sl, :], exp_sb[sl, :], recip[sl, :])
    nc.sync.dma_start(attn_d[sl, :], attn_sb[sl, :])
    nc.vector.tensor_copy(attn_bf[sl, :], attn_sb[sl, :])

    for ic in range(IC):
        pt = psum.tile([P, H], BF16, tag="tp", name=f"pt_at_{h}_{ic}")
        nc.tensor.transpose(
            pt[:], attn_bf[sl, ic * P : (ic + 1) * P], ident_bf[sl, r0 : r0 + H]
        )
        nc.vector.tensor_copy(attnT_bf[:, ic, sl], pt[:])

    # mix^T -> combined chunks 0..3
    for ac in range(AC):
        pm = psum.tile([P, H], F32, tag="sm", name=f"pm_{h}_{ac}")
        for ic in range(IC):
            nc.tensor.matmul(
                pm[:],
                ctx_bf[:, ic, ac * P : (ac + 1) * P],
                attnT_bf[:, ic, sl],
                start=(ic == 0),
                stop=(ic == IC - 1),
            )
        nc.vector.tensor_copy(combT_bf[:, ac, sl], pm[:])


def _final_project(nc, combT_bf, out_w_bf, ones_bf, outb_row_bf, out_sb, psum,
                   out_d):
    """out = tanh(combined @ out_w + out_b) for all 64 rows at once:
    M=64 x N=512 matmuls, bias applied as a rank-1 (K=1) accumulation."""
    po = psum.tile([OUT_LEN, DEC], F32, tag="mm", name="po_final")
    for cc in range(CC):
        nc.tensor.matmul(
            po[:], combT_bf[:, cc, :], out_w_bf[:, cc, :],
            start=(cc == 0), stop=False,
        )
    nc.tensor.matmul(po[:], ones_bf[:], outb_row_bf[:], start=False, stop=True)
    nc.scalar.activation(out_sb[:], po[:], AF.Tanh)
    nc.sync.dma_start(out_d[:], out_sb[:])


def _build_body(tc):
    nc = tc.nc

    # ---- DRAM I/O (per-core shard shapes) ----
    output_d = nc.dram_tensor("output", [OUT_LEN, DEC], F32, kind="ExternalInput").ap()
    context_d = nc.dram_tensor("context", [IN_LEN, ATTN], F32, kind="ExternalInput").ap()
    dec_w_d = nc.dram_tensor("dec_w", [DEC, DEC], F32, kind="ExternalInput").ap()
    dec_b_d = nc.dram_tensor("dec_b", [DEC, 1], F32, kind="ExternalInput").ap()
    attn_w_d = nc.dram_tensor("attn_w", [ATTN, DEC], F32, kind="ExternalInput").ap()
    attn_b_d = nc.dram_tensor("attn_b", [ATTN, 1], F32, kind="ExternalInput").ap()
    query_w_d = nc.dram_tensor("query_w", [DEC, 1], F32, kind="ExternalInput").ap()
    out_w_d = nc.dram_tensor("out_w", [ATTN + DEC, DEC], F32, kind="ExternalInput").ap()
    out_b_d = nc.dram_tensor("out_b", [DEC, 1], F32, kind="ExternalInput").ap()
    out_d = nc.dram_tensor("out", [OUT_LEN, DEC], F32, kind="ExternalOutput").ap()
    attn_d = nc.dram_tensor("attn", [OUT_LEN, IN_LEN], F32, kind="ExternalOutput").ap()

    from contextlib import ExitStack

    with ExitStack() as ctx:
        const = ctx.enter_context(tc.tile_pool(name="const", bufs=1))
        statics = ctx.enter_context(tc.tile_pool(name="statics", bufs=1))
        epool = ctx.enter_context(tc.tile_pool(name="epool", bufs=4))
        fpool = ctx.enter_context(tc.tile_pool(name="fpool", bufs=3))
        spool = ctx.enter_context(tc.tile_pool(name="spool", bufs=2))
        psum = ctx.enter_context(tc.tile_pool(name="psum", bufs=2, space="PSUM"))

        # ---------------- constants / small inputs ----------------
        ident = const.tile([P, P], F32)
        make_identity(nc, ident)
        ident_bf = const.tile([P, P], BF16)
        nc.vector.tensor_copy(ident_bf[:], ident[:])

        # HAM warmup: ~4us of real matmul activity on dummy data flips the
        # PE clock gate to 8/8 (2.4 GHz) before the real matmuls arrive.
        # (PE-transpose-mode does not count as HAM activity.)
        wu = psum.tile([P, P], F32, tag="mm")
        for _ in range(16):
            nc.tensor.matmul(wu[:], ident_bf[:], ident_bf[:], start=True, stop=True)

        attn_bias = const.tile([P, DC], F32)
        dec_bias = const.tile([P, DC], F32)
        qw_f = const.tile([P, DC], F32)
        qw_bf = const.tile([P, DC], BF16)
        for tile_, dram_ in ((attn_bias, attn_b_d), (dec_bias, dec_b_d),
                             (qw_f, query_w_d)):
            nc.scalar.dma_start(
                tile_[:], dram_.rearrange("(dc p) one -> p dc one", p=P)
            )
        nc.vector.tensor_copy(qw_bf[:], qw_f[:])

        ones_bf = const.tile([1, OUT_LEN], BF16)
        nc.vector.memset(ones_bf[:], 1.0)
        outb_row_f = const.tile([1, DEC], F32)
        nc.scalar.dma_start(outb_row_f[:], out_b_d.rearrange("d one -> one d"))
        outb_row_bf = const.tile([1, DEC], BF16)
        nc.vector.tensor_copy(outb_row_bf[:], outb_row_f[:])


        # ---------------- big input DMAs (split for queue parallelism) ----
        ctx_sb = statics.tile([P, IC, ATTN], F32)      # [i%, ic, a]
        attn_w_sb = statics.tile([P, AC, DEC], F32)    # [a%, ac, d]
        dec_w_sb = statics.tile([P, EC, DEC], F32)     # [e%, ec, d]
        output_sb = statics.tile([OUT_LEN, DEC], F32)  # [o, e]
        out_w_sb = statics.tile([P, CC, DEC], F32)     # [c%, cc, d]
        ctx_bf = statics.tile([P, IC, ATTN], BF16)
        attn_w_bf = statics.tile([P, AC, DEC], BF16)
        dec_w_bf = statics.tile([P, EC, DEC], BF16)
        out_w_bf = statics.tile([P, CC, DEC], BF16)
        output_bf = statics.tile([OUT_LEN, DEC], BF16)
        for ic in range(IC):
            nc.sync.dma_start(ctx_sb[:, ic, :], context_d[ic * P : (ic + 1) * P, :])
        nc.scalar.dma_start(output_sb[:], output_d[:])
        for ac in range(AC):
            nc.scalar.dma_start(attn_w_sb[:, ac, :], attn_w_d[ac * P : (ac + 1) * P, :])
        for ec in range(EC):
            nc.gpsimd.dma_start(dec_w_sb[:, ec, :], dec_w_d[ec * P : (ec + 1) * P, :])
        # bridge matmuls: keep the PE HAM-busy while DMAs land (paced by deps)
        for ic in range(IC):
            wub = psum.tile([P, ATTN], F32, tag="mm", name=f"wub_{ic}")
            nc.tensor.matmul(wub[:], ident[:], ctx_sb[:, ic, :], start=True, stop=True)
        for ic in range(IC):
            nc.vector.tensor_copy(ctx_bf[:, ic, :], ctx_sb[:, ic, :])
        nc.vector.tensor_copy(attn_w_bf[:], attn_w_sb[:])
        nc.vector.tensor_copy(output_bf[:], output_sb[:])
        nc.vector.tensor_copy(dec_w_bf[:], dec_w_sb[:])

        # ---------------- transposes: context^T (bf16), output^T ----------
        ctxT_bf = statics.tile([P, AC, IN_LEN], BF16)  # [a%, ac, i]
        for ic in range(IC):
            for ac in range(AC):
                pt = psum.tile([P, P], BF16, tag="tp", name=f"pt_c_{ic}_{ac}")
                nc.tensor.transpose(pt[:], ctx_bf[:, ic, ac * P : (ac + 1) * P], ident_bf[:])
                nc.vector.tensor_copy(ctxT_bf[:, ac, ic * P : (ic + 1) * P], pt[:])

        # combined^T [c%, cc, o]: chunks 0..3 = mix^T (later), 4..7 = output^T
        combT_bf = statics.tile([P, CC, OUT_LEN], BF16)
        for ec in range(EC):
            pt = psum.tile([P, OUT_LEN], BF16, tag="tp", name=f"pt_ot_{ec}")
            nc.tensor.transpose(
                pt[:], output_bf[0:OUT_LEN, ec * P : (ec + 1) * P],
                ident_bf[0:OUT_LEN, 0:OUT_LEN],
            )
            nc.vector.tensor_copy(combT_bf[:, EC + ec, :], pt[:])

        # ---------------- A^T and O^T (interleaved per d-chunk) ----------
        ATb = statics.tile([P, DC, IN_LEN], BF16)      # [d%, dc, i]
        OTb = statics.tile([P, DC, OUT_LEN], F32)      # [d%, dc, o]
        for dc in range(DC):
            pa = psum.tile([P, IN_LEN], F32, tag="mm", name=f"pa_{dc}")
            for ac in range(AC):
                nc.tensor.matmul(
                    pa[:],
                    attn_w_bf[:, ac, dc * P : (dc + 1) * P],
                    ctxT_bf[:, ac, :],
                    start=(ac == 0),
                    stop=(ac == AC - 1),
                )
            nc.vector.tensor_scalar_add(ATb[:, dc, :], pa[:], attn_bias[:, dc : dc + 1])
            po = psum.tile([P, OUT_LEN], F32, tag="sm", name=f"po_{dc}")
            for ec in range(EC):
                nc.tensor.matmul(
                    po[:],
                    dec_w_bf[:, ec, dc * P : (dc + 1) * P],
                    combT_bf[:, EC + ec, :],
                    start=(ec == 0),
                    stop=(ec == EC - 1),
                )
            nc.vector.tensor_scalar_add(OTb[:, dc, :], po[:], dec_bias[:, dc : dc + 1])

        # out_w lands during the main loop (needed first by epilogue half 0)
        for cc in range(CC):
            nc.gpsimd.dma_start(out_w_sb[:, cc, :], out_w_d[cc * P : (cc + 1) * P, :])

        # zero-padded stationary operands: QZ[:, dc, j] is [128, G] with
        # query_w[dc] in column j, zeros elsewhere -> matmul j deposits
        # scores for o_j into PSUM row j, rows != j accumulate zeros.
        QZ = const.tile([P, DC, G, G], BF16)
        nc.vector.memset(QZ[:], 0.0)
        for dc in range(DC):
            for j in range(G):
                nc.vector.tensor_copy(QZ[:, dc, j, j : j + 1], qw_bf[:, dc : dc + 1])

        # ---------------- main loop: tanh + q-reduction ----------------
        scores_sb = statics.tile([OUT_LEN, IN_LEN], F32)
        exp_sb = statics.tile([OUT_LEN, IN_LEN], F32)
        sums = statics.tile([OUT_LEN, 1], F32)
        recip = statics.tile([OUT_LEN, 1], F32)
        attn_sb = statics.tile([OUT_LEN, IN_LEN], F32)
        attn_bf = statics.tile([OUT_LEN, IN_LEN], BF16)
        attnT_bf = statics.tile([P, IC, OUT_LEN], BF16)
        out_sb = statics.tile([OUT_LEN, DEC], F32)

        sm_args = (ident_bf, scores_sb, exp_sb, sums, recip, attn_sb, attn_bf,
                   attnT_bf, ctx_bf, combT_bf, psum, attn_d)

        for og in range(NG):
            ps8 = psum.tile([G, IN_LEN], F32, tag="sc", name=f"ps8_{og}")
            for dc in range(DC):
                E = epool.tile([P, G, IN_LEN], BF16, tag="E", name=f"E_{og}_{dc}")
                for j in range(G):
                    o = og * G + j
                    nc.vector.tensor_scalar_add(
                        E[:, j, :], ATb[:, dc, :], OTb[:, dc, o : o + 1]
                    )
                Fc = fpool.tile([P, G, IN_LEN], BF16, tag="F", name=f"F_{og}_{dc}")
                nc.scalar.activation(Fc[:], E[:], AF.Tanh)
                for j in range(G):
                    nc.tensor.matmul(
                        ps8[:],
                        QZ[:, dc, j],
                        Fc[:, j],
                        start=(dc == 0 and j == 0),
                        stop=(dc == DC - 1 and j == G - 1),
                    )
            stage8 = spool.tile([G, IN_LEN], F32, tag="st", name=f"stage8_{og}")
            nc.vector.tensor_copy(stage8[:], ps8[:])
            nc.sync.dma_start(scores_sb[og * G : (og + 1) * G, :], stage8[:])

            if og < DC:
                # out_w bf16 casts, spread over the first groups (DVE slack)
                nc.vector.tensor_copy(out_w_bf[:, 2 * og, :], out_w_sb[:, 2 * og, :])
                nc.vector.tensor_copy(
                    out_w_bf[:, 2 * og + 1, :], out_w_sb[:, 2 * og + 1, :]
                )

            if og == NG // 2:
                # rows 0..31 complete since og 3: their softmax + mix runs
                # under og 5..7 (placed here so the ACT stream never blocks)
                _epilogue_softmax_mix(nc, 0, *sm_args)

        # keep the PE warm across the softmax wait before the h1 mix
        for k in range(16):
            wut = psum.tile([P, P], F32, tag="mm", name=f"wut_{k}")
            nc.tensor.matmul(wut[:], ident_bf[:], ident_bf[:], start=True, stop=True)

        _epilogue_softmax_mix(nc, 1, *sm_args)
        _final_project(nc, combT_bf, out_w_bf, ones_bf, outb_row_bf, out_sb,
                       psum, out_d)


_CACHE = {}


def build_nc():
    if "nc" in _CACHE:
        return _CACHE["nc"]
    nc = bacc.Bacc(
        "TRN2",
        target_bir_lowering=False,
        debug=False,
        num_devices=N_CORES,
    )
    with tile.TileContext(nc) as tc:
        _build_body(tc)
    nc.compile()
    _CACHE["nc"] = nc
    return nc


def kernel(**inputs):
    nc = build_nc()

    f = lambda k: np.ascontiguousarray(np.asarray(inputs[k], dtype=np.float32))
    output = f("output")
    context = f("context")
    shared = {
        "dec_w": f("dec_w"),
        "dec_b": f("dec_b").reshape(DEC, 1),
        "attn_w": f("attn_w"),
        "attn_b": f("attn_b").reshape(ATTN, 1),
        "query_w": f("query_w").reshape(DEC, 1),
        "out_w": f("out_w"),
        "out_b": f("out_b").reshape(DEC, 1),
    }
    in_maps = []
    for b in range(N_CORES):
        m = dict(shared)
        m["output"] = np.ascontiguousarray(output[b])
        m["context"] = np.ascontiguousarray(context[b])
        in_maps.append(m)

    res = bass_utils.run_bass_kernel_spmd(nc, in_maps, core_ids=list(range(N_CORES)))
    _CACHE["last_results"] = res
    out = np.stack([res.results[b]["out"] for b in range(N_CORES)])
    attn = np.stack([res.results[b]["attn"] for b in range(N_CORES)])
    return out, attn


# revision 17
# speedup vs baseline: 1.0205x; 1.0205x over previous
"""Bass/Tile Trainium2 kernel for additive (Bahdanau/'cat') attention.

Problem (per batch b):
  A[i,d]      = sum_a context[i,a] * attn_w[a,d] + attn_b[d]
  O[o,d]      = sum_e output[o,e]  * dec_w[e,d]  + dec_b[d]
  scores[o,i] = sum_d query_w[d] * tanh(A[i,d] + O[o,d])   (+query_b: softmax-invariant)
  attn        = softmax_i(scores)
  mix[o,a]    = sum_i attn[o,i] * context[i,a]
  out[o,d]    = tanh([mix | output] @ out_w + out_b)

Sharding: pure data-parallel over batch, B=8 -> one batch per NeuronCore,
weights broadcast, no collectives.

Per-core structure:
  * A^T [d,i] and O^T [d,o] kept with d on partitions so the broadcast add
    A^T + O^T[:,o] is a DVE tensor_scalar (per-partition scalar), in bf16.
  * tanh batched 8 o's per ACT instruction (free dim 4096); d-chunk-outer
    so the PE gets matmul work after every ACT chunk (keeps HAM warm).
  * q-reduction over d on the PE with zero-padded stationary operand:
    lhsT QZ[:,dc,j] is [128,8] holding query_w in column j -> all 32
    matmuls of a group accumulate into ONE [8,512] PSUM bank; one cheap
    8-row DVE copy + SBUF->SBUF DMA scatters rows into scores.
  * softmax/mix/out epilogue computed in row-halves (0:32 during groups
    4..7, 32:64 at the end) to shorten the serial tail.
"""

import numpy as np

import concourse.bass as bass
import concourse.tile as tile
import concourse.bass_utils as bass_utils
from concourse import bacc, mybir
from concourse.masks import make_identity

B, OUT_LEN, IN_LEN, DEC, ATTN = 8, 64, 512, 512, 512
P = 128
F32 = mybir.dt.float32
BF16 = mybir.dt.bfloat16
AF = mybir.ActivationFunctionType

G = 8                     # o's per tanh group
NG = OUT_LEN // G         # 8 groups
DC = DEC // P             # 4 d-chunks
AC = ATTN // P            # 4 a-chunks
IC = IN_LEN // P          # 4 i-chunks
EC = DEC // P             # 4 e-chunks (decoder feature)
CC = (ATTN + DEC) // P    # 8 combined chunks
H = OUT_LEN // 2          # row half

N_CORES = 8


def _epilogue_softmax_mix(nc, h, ident_bf, scores_sb, exp_sb, sums, recip,
                          attn_sb, attn_bf, attnT_bf, ctx_bf, combT_bf, psum,
                          attn_d):
    """softmax + attn^T + mix for rows h*32..h*32+31 (all-bf16 matmuls)."""
    r0 = h * H
    sl = slice(r0, r0 + H)
    nc.scalar.activation(exp_sb[sl, :], scores_sb[sl, :], AF.Exp, accum_out=sums[sl, :])
    nc.vector.reciprocal(recip[sl, :], sums[sl, :])
    nc.vector.tensor_scalar_mul(attn_sb[sl, :], exp_sb[sl, :], recip[sl, :])
    nc.sync.dma_start(attn_d[sl, :], attn_sb[sl, :])
    nc.vector.tensor_copy(attn_bf[sl, :], attn_sb[sl, :])

    for ic in range(IC):
        pt = psum.tile([P, H], BF16, tag="tp", name=f"pt_at_{h}_{ic}")
        nc.tensor.transpose(
            pt[:], attn_bf[sl, ic * P : (ic + 1) * P], ident_bf[sl, r0 : r0 + H]
        )
        nc.vector.tensor_copy(attnT_bf[:, ic, sl], pt[:])

    # mix^T -> combined chunks 0..3
    for ac in range(AC):
        pm = psum.tile([P, H], F32, tag="sm", name=f"pm_{h}_{ac}")
        for ic in range(IC):
            nc.tensor.matmul(
                pm[:],
                ctx_bf[:, ic, ac * P : (ac + 1) * P],
                attnT_bf[:, ic, sl],
                start=(ic == 0),
                stop=(ic == IC - 1),
            )
        nc.vector.tensor_copy(combT_bf[:, ac, sl], pm[:])


def _final_project(nc, combT_bf, out_w_bf, ones_bf, outb_row_bf, out_sb, psum,
                   out_d):
    """out = tanh(combined @ out_w + out_b) for all 64 rows at once:
    M=64 x N=512 matmuls, bias applied as a rank-1 (K=1) accumulation."""
    po = psum.tile([OUT_LEN, DEC], F32, tag="mm", name="po_final")
    for cc in range(CC):
        nc.tensor.matmul(
            po[:], combT_bf[:, cc, :], out_w_bf[:, cc, :],
            start=(cc == 0), stop=False,
        )
    nc.tensor.matmul(po[:], ones_bf[:], outb_row_bf[:], start=False, stop=True)
    nc.scalar.activation(out_sb[:], po[:], AF.Tanh)
    nc.sync.dma_start(out_d[:], out_sb[:])


def _build_body(tc):
    nc = tc.nc

    # ---- DRAM I/O (per-core shard shapes) ----
    output_d = nc.dram_tensor("output", [OUT_LEN, DEC], F32, kind="ExternalInput").ap()
    context_d = nc.dram_tensor("context", [IN_LEN, ATTN], F32, kind="ExternalInput").ap()
    dec_w_d = nc.dram_tensor("dec_w", [DEC, DEC], F32, kind="ExternalInput").ap()
    dec_b_d = nc.dram_tensor("dec_b", [DEC, 1], F32, kind="ExternalInput").ap()
    attn_w_d = nc.dram_tensor("attn_w", [ATTN, DEC], F32, kind="ExternalInput").ap()
    attn_b_d = nc.dram_tensor("attn_b", [ATTN, 1], F32, kind="ExternalInput").ap()
    query_w_d = nc.dram_tensor("query_w", [DEC, 1], F32, kind="ExternalInput").ap()
    out_w_d = nc.dram_tensor("out_w", [ATTN + DEC, DEC], F32, kind="ExternalInput").ap()
    out_b_d = nc.dram_tensor("out_b", [DEC, 1], F32, kind="ExternalInput").ap()
    out_d = nc.dram_tensor("out", [OUT_LEN, DEC], F32, kind="ExternalOutput").ap()
    attn_d = nc.dram_tensor("attn", [OUT_LEN, IN_LEN], F32, kind="ExternalOutput").ap()

    from contextlib import ExitStack

    with ExitStack() as ctx:
        const = ctx.enter_context(tc.tile_pool(name="const", bufs=1))
        statics = ctx.enter_context(tc.tile_pool(name="statics", bufs=1))
        epool = ctx.enter_context(tc.tile_pool(name="epool", bufs=4))
        fpool = ctx.enter_context(tc.tile_pool(name="fpool", bufs=3))
        spool = ctx.enter_context(tc.tile_pool(name="spool", bufs=2))
        psum = ctx.enter_context(tc.tile_pool(name="psum", bufs=2, space="PSUM"))

        # ---------------- constants / small inputs ----------------
        ident = const.tile([P, P], F32)
        make_identity(nc, ident)
        ident_bf = const.tile([P, P], BF16)
        nc.vector.tensor_copy(ident_bf[:], ident[:])

        # HAM warmup: ~4us of real matmul activity on dummy data flips the
        # PE clock gate to 8/8 (2.4 GHz) before the real matmuls arrive.
        # (PE-transpose-mode does not count as HAM activity.)
        wu = psum.tile([P, P], F32, tag="mm")
        for _ in range(16):
            nc.tensor.matmul(wu[:], ident_bf[:], ident_bf[:], start=True, stop=True)

        attn_bias = const.tile([P, DC], F32)
        dec_bias = const.tile([P, DC], F32)
        qw_f = const.tile([P, DC], F32)
        qw_bf = const.tile([P, DC], BF16)
        for tile_, dram_ in ((attn_bias, attn_b_d), (dec_bias, dec_b_d),
                             (qw_f, query_w_d)):
            nc.scalar.dma_start(
                tile_[:], dram_.rearrange("(dc p) one -> p dc one", p=P)
            )
        nc.vector.tensor_copy(qw_bf[:], qw_f[:])

        ones_bf = const.tile([1, OUT_LEN], BF16)
        nc.vector.memset(ones_bf[:], 1.0)
        outb_row_f = const.tile([1, DEC], F32)
        nc.scalar.dma_start(outb_row_f[:], out_b_d.rearrange("d one -> one d"))
        outb_row_bf = const.tile([1, DEC], BF16)
        nc.vector.tensor_copy(outb_row_bf[:], outb_row_f[:])


        # ---------------- big input DMAs (split for queue parallelism) ----
        ctx_sb = statics.tile([P, IC, ATTN], F32)      # [i%, ic, a]
        attn_w_sb = statics.tile([P, AC, DEC], F32)    # [a%, ac, d]
        dec_w_sb = statics.tile([P, EC, DEC], F32)     # [e%, ec, d]
        output_sb = statics.tile([OUT_LEN, DEC], F32)  # [o, e]
        out_w_sb = statics.tile([P, CC, DEC], F32)     # [c%, cc, d]
        ctx_bf = statics.tile([P, IC, ATTN], BF16)
        attn_w_bf = statics.tile([P, AC, DEC], BF16)
        dec_w_bf = statics.tile([P, EC, DEC], BF16)
        out_w_bf = statics.tile([P, CC, DEC], BF16)
        output_bf = statics.tile([OUT_LEN, DEC], BF16)
        for ic in range(IC):
            nc.sync.dma_start(ctx_sb[:, ic, :], context_d[ic * P : (ic + 1) * P, :])
        nc.scalar.dma_start(output_sb[:], output_d[:])
        for ac in range(AC):
            nc.scalar.dma_start(attn_w_sb[:, ac, :], attn_w_d[ac * P : (ac + 1) * P, :])
        for ec in range(EC):
            nc.sync.dma_start(dec_w_sb[:, ec, :], dec_w_d[ec * P : (ec + 1) * P, :])
        # bridge matmuls: keep the PE HAM-busy while DMAs land (paced by deps)
        for ic in range(IC):
            wub = psum.tile([P, ATTN], F32, tag="mm", name=f"wub_{ic}")
            nc.tensor.matmul(wub[:], ident[:], ctx_sb[:, ic, :], start=True, stop=True)
        for ic in range(IC):
            nc.vector.tensor_copy(ctx_bf[:, ic, :], ctx_sb[:, ic, :])
        for ac in range(AC):
            nc.vector.tensor_copy(attn_w_bf[:, ac, :], attn_w_sb[:, ac, :])
        nc.vector.tensor_copy(output_bf[:], output_sb[:])
        for ec in range(EC):
            nc.vector.tensor_copy(dec_w_bf[:, ec, :], dec_w_sb[:, ec, :])

        # ---------------- transposes: context^T (bf16), output^T ----------
        ctxT_bf = statics.tile([P, AC, IN_LEN], BF16)  # [a%, ac, i]
        for ic in range(IC):
            for ac in range(AC):
                pt = psum.tile([P, P], BF16, tag="tp", name=f"pt_c_{ic}_{ac}")
                nc.tensor.transpose(pt[:], ctx_bf[:, ic, ac * P : (ac + 1) * P], ident_bf[:])
                nc.vector.tensor_copy(ctxT_bf[:, ac, ic * P : (ic + 1) * P], pt[:])

        # combined^T [c%, cc, o]: chunks 0..3 = mix^T (later), 4..7 = output^T
        combT_bf = statics.tile([P, CC, OUT_LEN], BF16)
        for ec in range(EC):
            pt = psum.tile([P, OUT_LEN], BF16, tag="tp", name=f"pt_ot_{ec}")
            nc.tensor.transpose(
                pt[:], output_bf[0:OUT_LEN, ec * P : (ec + 1) * P],
                ident_bf[0:OUT_LEN, 0:OUT_LEN],
            )
            nc.vector.tensor_copy(combT_bf[:, EC + ec, :], pt[:])

        # ---------------- O^T then A^T ----------------
        OTb = statics.tile([P, DC, OUT_LEN], F32)      # [d%, dc, o]
        for dc in range(DC):
            po = psum.tile([P, OUT_LEN], F32, tag="sm", name=f"po_{dc}")
            for ec in range(EC):
                nc.tensor.matmul(
                    po[:],
                    dec_w_bf[:, ec, dc * P : (dc + 1) * P],
                    combT_bf[:, EC + ec, :],
                    start=(ec == 0),
                    stop=(ec == EC - 1),
                )
            nc.vector.tensor_scalar_add(OTb[:, dc, :], po[:], dec_bias[:, dc : dc + 1])

        ATb = statics.tile([P, DC, IN_LEN], BF16)      # [d%, dc, i]
        for dc in range(DC):
            pa = psum.tile([P, IN_LEN], F32, tag="mm", name=f"pa_{dc}")
            for ac in range(AC):
                nc.tensor.matmul(
                    pa[:],
                    attn_w_bf[:, ac, dc * P : (dc + 1) * P],
                    ctxT_bf[:, ac, :],
                    start=(ac == 0),
                    stop=(ac == AC - 1),
                )
            nc.vector.tensor_scalar_add(ATb[:, dc, :], pa[:], attn_bias[:, dc : dc + 1])

        # out_w lands during the main loop (needed first by epilogue half 0)
        for cc in range(CC):
            nc.sync.dma_start(out_w_sb[:, cc, :], out_w_d[cc * P : (cc + 1) * P, :])

        # zero-padded stationary operands: QZ[:, dc, j] is [128, G] with
        # query_w[dc] in column j, zeros elsewhere -> matmul j deposits
        # scores for o_j into PSUM row j, rows != j accumulate zeros.
        QZ = const.tile([P, DC, G, G], BF16)
        nc.vector.memset(QZ[:], 0.0)
        for dc in range(DC):
            for j in range(G):
                nc.vector.tensor_copy(QZ[:, dc, j, j : j + 1], qw_bf[:, dc : dc + 1])

        # ---------------- main loop: tanh + q-reduction ----------------
        scores_sb = statics.tile([OUT_LEN, IN_LEN], F32)
        exp_sb = statics.tile([OUT_LEN, IN_LEN], F32)
        sums = statics.tile([OUT_LEN, 1], F32)
        recip = statics.tile([OUT_LEN, 1], F32)
        attn_sb = statics.tile([OUT_LEN, IN_LEN], F32)
        attn_bf = statics.tile([OUT_LEN, IN_LEN], BF16)
        attnT_bf = statics.tile([P, IC, OUT_LEN], BF16)
        out_sb = statics.tile([OUT_LEN, DEC], F32)

        sm_args = (ident_bf, scores_sb, exp_sb, sums, recip, attn_sb, attn_bf,
                   attnT_bf, ctx_bf, combT_bf, psum, attn_d)

        for og in range(NG):
            ps8 = psum.tile([G, IN_LEN], F32, tag="sc", name=f"ps8_{og}")
            for dc in range(DC):
                E = epool.tile([P, G, IN_LEN], BF16, tag="E", name=f"E_{og}_{dc}")
                for j in range(G):
                    o = og * G + j
                    nc.vector.tensor_scalar_add(
                        E[:, j, :], ATb[:, dc, :], OTb[:, dc, o : o + 1]
                    )
                Fc = fpool.tile([P, G, IN_LEN], BF16, tag="F", name=f"F_{og}_{dc}")
                nc.scalar.activation(Fc[:], E[:], AF.Tanh)
                for j in range(G):
                    nc.tensor.matmul(
                        ps8[:],
                        QZ[:, dc, j],
                        Fc[:, j],
                        start=(dc == 0 and j == 0),
                        stop=(dc == DC - 1 and j == G - 1),
                    )
            stage8 = spool.tile([G, IN_LEN], F32, tag="st", name=f"stage8_{og}")
            nc.vector.tensor_copy(stage8[:], ps8[:])
            nc.sync.dma_start(scores_sb[og * G : (og + 1) * G, :], stage8[:])

            if og < DC:
                # out_w bf16 casts, spread over the first groups (DVE slack)
                nc.vector.tensor_copy(out_w_bf[:, 2 * og, :], out_w_sb[:, 2 * og, :])
                nc.vector.tensor_copy(
                    out_w_bf[:, 2 * og + 1, :], out_w_sb[:, 2 * og + 1, :]
                )

            if og == NG // 2:
                # rows 0..31 complete since og 3: their softmax + mix runs
                # under og 5..7 (placed here so the ACT stream never blocks)
                _epilogue_softmax_mix(nc, 0, *sm_args)

        # keep the PE warm across the softmax wait before the h1 mix
        for k in range(16):
            wut = psum.tile([P, P], F32, tag="mm", name=f"wut_{k}")
            nc.tensor.matmul(wut[:], ident_bf[:], ident_bf[:], start=True, stop=True)

        _epilogue_softmax_mix(nc, 1, *sm_args)
        _final_project(nc, combT_bf, out_w_bf, ones_bf, outb_row_bf, out_sb,
                       psum, out_d)


_CACHE = {}


def build_nc():
    if "nc" in _CACHE:
        return _CACHE["nc"]
    nc = bacc.Bacc(
        "TRN2",
        target_bir_lowering=False,
        debug=False,
        num_devices=N_CORES,
    )
    with tile.TileContext(nc) as tc:
        _build_body(tc)
    nc.compile()
    _CACHE["nc"] = nc
    return nc


def kernel(**inputs):
    nc = build_nc()

    f = lambda k: np.ascontiguousarray(np.asarray(inputs[k], dtype=np.float32))
    output = f("output")
    context = f("context")
    shared = {
        "dec_w": f("dec_w"),
        "dec_b": f("dec_b").reshape(DEC, 1),
        "attn_w": f("attn_w"),
        "attn_b": f("attn_b").reshape(ATTN, 1),
        "query_w": f("query_w").reshape(DEC, 1),
        "out_w": f("out_w"),
        "out_b": f("out_b").reshape(DEC, 1),
    }
    in_maps = []
    for b in range(N_CORES):
        m = dict(shared)
        m["output"] = np.ascontiguousarray(output[b])
        m["context"] = np.ascontiguousarray(context[b])
        in_maps.append(m)

    res = bass_utils.run_bass_kernel_spmd(nc, in_maps, core_ids=list(range(N_CORES)))
    _CACHE["last_results"] = res
    out = np.stack([res.results[b]["out"] for b in range(N_CORES)])
    attn = np.stack([res.results[b]["attn"] for b in range(N_CORES)])
    return out, attn


# revision 18
# speedup vs baseline: 1.0259x; 1.0054x over previous
"""Bass/Tile Trainium2 kernel for additive (Bahdanau/'cat') attention.

Problem (per batch b):
  A[i,d]      = sum_a context[i,a] * attn_w[a,d] + attn_b[d]
  O[o,d]      = sum_e output[o,e]  * dec_w[e,d]  + dec_b[d]
  scores[o,i] = sum_d query_w[d] * tanh(A[i,d] + O[o,d])   (+query_b: softmax-invariant)
  attn        = softmax_i(scores)
  mix[o,a]    = sum_i attn[o,i] * context[i,a]
  out[o,d]    = tanh([mix | output] @ out_w + out_b)

Sharding: pure data-parallel over batch, B=8 -> one batch per NeuronCore,
weights broadcast, no collectives.

Per-core structure:
  * A^T [d,i] and O^T [d,o] kept with d on partitions so the broadcast add
    A^T + O^T[:,o] is a DVE tensor_scalar (per-partition scalar), in bf16.
  * tanh batched 8 o's per ACT instruction (free dim 4096); d-chunk-outer
    so the PE gets matmul work after every ACT chunk (keeps HAM warm).
  * q-reduction over d on the PE with zero-padded stationary operand:
    lhsT QZ[:,dc,j] is [128,8] holding query_w in column j -> all 32
    matmuls of a group accumulate into ONE [8,512] PSUM bank; one cheap
    8-row DVE copy + SBUF->SBUF DMA scatters rows into scores.
  * softmax/mix/out epilogue computed in row-halves (0:32 during groups
    4..7, 32:64 at the end) to shorten the serial tail.
"""

import numpy as np

import concourse.bass as bass
import concourse.tile as tile
import concourse.bass_utils as bass_utils
from concourse import bacc, mybir
from concourse.masks import make_identity

B, OUT_LEN, IN_LEN, DEC, ATTN = 8, 64, 512, 512, 512
P = 128
F32 = mybir.dt.float32
BF16 = mybir.dt.bfloat16
AF = mybir.ActivationFunctionType

G = 8                     # o's per tanh group
NG = OUT_LEN // G         # 8 groups
DC = DEC // P             # 4 d-chunks
AC = ATTN // P            # 4 a-chunks
IC = IN_LEN // P          # 4 i-chunks
EC = DEC // P             # 4 e-chunks (decoder feature)
CC = (ATTN + DEC) // P    # 8 combined chunks
H = OUT_LEN // 2          # row half

N_CORES = 8


def _epilogue_softmax_mix(nc, h, ident_bf, scores_sb, exp_sb, sums, recip,
                          attn_sb, attn_bf, attnT_bf, ctx_bf, combT_bf, psum,
                          attn_d):
    """softmax + attn^T + mix for rows h*32..h*32+31 (all-bf16 matmuls)."""
    r0 = h * H
    sl = slice(r0, r0 + H)
    nc.scalar.activation(exp_sb[sl, :], scores_sb[sl, :], AF.Exp, accum_out=sums[sl, :])
    nc.vector.reciprocal(recip[sl, :], sums[sl, :])
    nc.vector.tensor_scalar_mul(attn_sb[sl, :], exp_sb[sl, :], recip[sl, :])
    nc.sync.dma_start(attn_d[sl, :], attn_sb[sl, :])
    nc.vector.tensor_copy(attn_bf[sl, :], attn_sb[sl, :])

    for ic in range(IC):
        pt = psum.tile([P, H], BF16, tag="tp", name=f"pt_at_{h}_{ic}")
        nc.tensor.transpose(
            pt[:], attn_bf[sl, ic * P : (ic + 1) * P], ident_bf[sl, r0 : r0 + H]
        )
        nc.vector.tensor_copy(attnT_bf[:, ic, sl], pt[:])

    # mix^T -> combined chunks 0..3
    for ac in range(AC):
        pm = psum.tile([P, H], F32, tag="sm", name=f"pm_{h}_{ac}")
        for ic in range(IC):
            nc.tensor.matmul(
                pm[:],
                ctx_bf[:, ic, ac * P : (ac + 1) * P],
                attnT_bf[:, ic, sl],
                start=(ic == 0),
                stop=(ic == IC - 1),
            )
        nc.vector.tensor_copy(combT_bf[:, ac, sl], pm[:])


def _final_project(nc, combT_bf, out_w_bf, ones_bf, outb_row_bf, out_sb, psum,
                   out_d):
    """out = tanh(combined @ out_w + out_b) for all 64 rows at once:
    M=64 x N=512 matmuls, bias applied as a rank-1 (K=1) accumulation."""
    po = psum.tile([OUT_LEN, DEC], F32, tag="mm", name="po_final")
    for cc in range(CC):
        nc.tensor.matmul(
            po[:], combT_bf[:, cc, :], out_w_bf[:, cc, :],
            start=(cc == 0), stop=False,
        )
    nc.tensor.matmul(po[:], ones_bf[:], outb_row_bf[:], start=False, stop=True)
    nc.scalar.activation(out_sb[:], po[:], AF.Tanh)
    nc.sync.dma_start(out_d[:], out_sb[:])


def _build_body(tc):
    nc = tc.nc

    # ---- DRAM I/O (per-core shard shapes) ----
    output_d = nc.dram_tensor("output", [OUT_LEN, DEC], F32, kind="ExternalInput").ap()
    context_d = nc.dram_tensor("context", [IN_LEN, ATTN], F32, kind="ExternalInput").ap()
    dec_w_d = nc.dram_tensor("dec_w", [DEC, DEC], F32, kind="ExternalInput").ap()
    dec_b_d = nc.dram_tensor("dec_b", [DEC, 1], F32, kind="ExternalInput").ap()
    attn_w_d = nc.dram_tensor("attn_w", [ATTN, DEC], F32, kind="ExternalInput").ap()
    attn_b_d = nc.dram_tensor("attn_b", [ATTN, 1], F32, kind="ExternalInput").ap()
    query_w_d = nc.dram_tensor("query_w", [DEC, 1], F32, kind="ExternalInput").ap()
    out_w_d = nc.dram_tensor("out_w", [ATTN + DEC, DEC], F32, kind="ExternalInput").ap()
    out_b_d = nc.dram_tensor("out_b", [DEC, 1], F32, kind="ExternalInput").ap()
    out_d = nc.dram_tensor("out", [OUT_LEN, DEC], F32, kind="ExternalOutput").ap()
    attn_d = nc.dram_tensor("attn", [OUT_LEN, IN_LEN], F32, kind="ExternalOutput").ap()

    from contextlib import ExitStack

    with ExitStack() as ctx:
        const = ctx.enter_context(tc.tile_pool(name="const", bufs=1))
        statics = ctx.enter_context(tc.tile_pool(name="statics", bufs=1))
        epool = ctx.enter_context(tc.tile_pool(name="epool", bufs=4))
        fpool = ctx.enter_context(tc.tile_pool(name="fpool", bufs=3))
        spool = ctx.enter_context(tc.tile_pool(name="spool", bufs=2))
        psum = ctx.enter_context(tc.tile_pool(name="psum", bufs=2, space="PSUM"))

        # ---------------- constants / small inputs ----------------
        ident = const.tile([P, P], F32)
        make_identity(nc, ident)
        ident_bf = const.tile([P, P], BF16)
        nc.vector.tensor_copy(ident_bf[:], ident[:])

        # HAM warmup: ~4us of real matmul activity on dummy data flips the
        # PE clock gate to 8/8 (2.4 GHz) before the real matmuls arrive.
        # (PE-transpose-mode does not count as HAM activity.)
        wu = psum.tile([P, P], F32, tag="mm")
        for _ in range(16):
            nc.tensor.matmul(wu[:], ident_bf[:], ident_bf[:], start=True, stop=True)

        attn_bias = const.tile([P, DC], F32)
        dec_bias = const.tile([P, DC], F32)
        qw_f = const.tile([P, DC], F32)
        qw_bf = const.tile([P, DC], BF16)
        for tile_, dram_ in ((attn_bias, attn_b_d), (dec_bias, dec_b_d),
                             (qw_f, query_w_d)):
            nc.scalar.dma_start(
                tile_[:], dram_.rearrange("(dc p) one -> p dc one", p=P)
            )
        nc.vector.tensor_copy(qw_bf[:], qw_f[:])

        ones_bf = const.tile([1, OUT_LEN], BF16)
        nc.vector.memset(ones_bf[:], 1.0)
        outb_row_f = const.tile([1, DEC], F32)
        nc.scalar.dma_start(outb_row_f[:], out_b_d.rearrange("d one -> one d"))
        outb_row_bf = const.tile([1, DEC], BF16)
        nc.vector.tensor_copy(outb_row_bf[:], outb_row_f[:])


        # ---------------- big input DMAs (split for queue parallelism) ----
        ctx_sb = statics.tile([P, IC, ATTN], F32)      # [i%, ic, a]
        attn_w_sb = statics.tile([P, AC, DEC], F32)    # [a%, ac, d]
        dec_w_sb = statics.tile([P, EC, DEC], F32)     # [e%, ec, d]
        output_sb = statics.tile([OUT_LEN, DEC], F32)  # [o, e]
        out_w_sb = statics.tile([P, CC, DEC], F32)     # [c%, cc, d]
        ctx_bf = statics.tile([P, IC, ATTN], BF16)
        attn_w_bf = statics.tile([P, AC, DEC], BF16)
        dec_w_bf = statics.tile([P, EC, DEC], BF16)
        out_w_bf = statics.tile([P, CC, DEC], BF16)
        output_bf = statics.tile([OUT_LEN, DEC], BF16)
        for ic in range(IC):
            nc.sync.dma_start(ctx_sb[:, ic, :], context_d[ic * P : (ic + 1) * P, :])
        nc.scalar.dma_start(output_sb[:], output_d[:])
        for ac in range(AC):
            nc.scalar.dma_start(attn_w_sb[:, ac, :], attn_w_d[ac * P : (ac + 1) * P, :])
        for ec in range(EC):
            nc.sync.dma_start(dec_w_sb[:, ec, :], dec_w_d[ec * P : (ec + 1) * P, :])
        # bridge matmuls: keep the PE HAM-busy while DMAs land (paced by deps)
        for ic in range(IC):
            wub = psum.tile([P, ATTN], F32, tag="mm", name=f"wub_{ic}")
            nc.tensor.matmul(wub[:], ident[:], ctx_sb[:, ic, :], start=True, stop=True)
        for ic in range(IC):
            nc.vector.tensor_copy(ctx_bf[:, ic, :], ctx_sb[:, ic, :])
        nc.vector.tensor_copy(output_bf[:], output_sb[:])
        for ac in range(AC):
            nc.vector.tensor_copy(attn_w_bf[:, ac, :], attn_w_sb[:, ac, :])
        for ec in range(EC):
            nc.vector.tensor_copy(dec_w_bf[:, ec, :], dec_w_sb[:, ec, :])

        # ---------------- transposes: context^T (bf16), output^T ----------
        ctxT_bf = statics.tile([P, AC, IN_LEN], BF16)  # [a%, ac, i]
        for ic in range(IC):
            for ac in range(AC):
                pt = psum.tile([P, P], BF16, tag="tp", name=f"pt_c_{ic}_{ac}")
                nc.tensor.transpose(pt[:], ctx_bf[:, ic, ac * P : (ac + 1) * P], ident_bf[:])
                nc.vector.tensor_copy(ctxT_bf[:, ac, ic * P : (ic + 1) * P], pt[:])

        # combined^T [c%, cc, o]: chunks 0..3 = mix^T (later), 4..7 = output^T
        combT_bf = statics.tile([P, CC, OUT_LEN], BF16)
        for ec in range(EC):
            pt = psum.tile([P, OUT_LEN], BF16, tag="tp", name=f"pt_ot_{ec}")
            nc.tensor.transpose(
                pt[:], output_bf[0:OUT_LEN, ec * P : (ec + 1) * P],
                ident_bf[0:OUT_LEN, 0:OUT_LEN],
            )
            nc.vector.tensor_copy(combT_bf[:, EC + ec, :], pt[:])

        # ---------------- A^T then O^T ----------------
        ATb = statics.tile([P, DC, IN_LEN], BF16)      # [d%, dc, i]
        for dc in range(DC):
            pa = psum.tile([P, IN_LEN], F32, tag="mm", name=f"pa_{dc}")
            for ac in range(AC):
                nc.tensor.matmul(
                    pa[:],
                    attn_w_bf[:, ac, dc * P : (dc + 1) * P],
                    ctxT_bf[:, ac, :],
                    start=(ac == 0),
                    stop=(ac == AC - 1),
                )
            nc.vector.tensor_scalar_add(ATb[:, dc, :], pa[:], attn_bias[:, dc : dc + 1])

        OTb = statics.tile([P, DC, OUT_LEN], F32)      # [d%, dc, o]
        for dc in range(DC):
            po = psum.tile([P, OUT_LEN], F32, tag="sm", name=f"po_{dc}")
            for ec in range(EC):
                nc.tensor.matmul(
                    po[:],
                    dec_w_bf[:, ec, dc * P : (dc + 1) * P],
                    combT_bf[:, EC + ec, :],
                    start=(ec == 0),
                    stop=(ec == EC - 1),
                )
            nc.vector.tensor_scalar_add(OTb[:, dc, :], po[:], dec_bias[:, dc : dc + 1])

        # out_w lands during the main loop (needed first by epilogue half 0)
        for cc in range(CC):
            nc.sync.dma_start(out_w_sb[:, cc, :], out_w_d[cc * P : (cc + 1) * P, :])

        # zero-padded stationary operands: QZ[:, dc, j] is [128, G] with
        # query_w[dc] in column j, zeros elsewhere -> matmul j deposits
        # scores for o_j into PSUM row j, rows != j accumulate zeros.
        QZ = const.tile([P, DC, G, G], BF16)
        nc.vector.memset(QZ[:], 0.0)
        for dc in range(DC):
            for j in range(G):
                nc.vector.tensor_copy(QZ[:, dc, j, j : j + 1], qw_bf[:, dc : dc + 1])

        # ---------------- main loop: tanh + q-reduction ----------------
        scores_sb = statics.tile([OUT_LEN, IN_LEN], F32)
        exp_sb = statics.tile([OUT_LEN, IN_LEN], F32)
        sums = statics.tile([OUT_LEN, 1], F32)
        recip = statics.tile([OUT_LEN, 1], F32)
        attn_sb = statics.tile([OUT_LEN, IN_LEN], F32)
        attn_bf = statics.tile([OUT_LEN, IN_LEN], BF16)
        attnT_bf = statics.tile([P, IC, OUT_LEN], BF16)
        out_sb = statics.tile([OUT_LEN, DEC], F32)

        sm_args = (ident_bf, scores_sb, exp_sb, sums, recip, attn_sb, attn_bf,
                   attnT_bf, ctx_bf, combT_bf, psum, attn_d)

        for og in range(NG):
            ps8 = psum.tile([G, IN_LEN], F32, tag="sc", name=f"ps8_{og}")
            for dc in range(DC):
                E = epool.tile([P, G, IN_LEN], BF16, tag="E", name=f"E_{og}_{dc}")
                for j in range(G):
                    o = og * G + j
                    nc.vector.tensor_scalar_add(
                        E[:, j, :], ATb[:, dc, :], OTb[:, dc, o : o + 1]
                    )
                Fc = fpool.tile([P, G, IN_LEN], BF16, tag="F", name=f"F_{og}_{dc}")
                nc.scalar.activation(Fc[:], E[:], AF.Tanh)
                for j in range(G):
                    nc.tensor.matmul(
                        ps8[:],
                        QZ[:, dc, j],
                        Fc[:, j],
                        start=(dc == 0 and j == 0),
                        stop=(dc == DC - 1 and j == G - 1),
                    )
            stage8 = spool.tile([G, IN_LEN], F32, tag="st", name=f"stage8_{og}")
            nc.vector.tensor_copy(stage8[:], ps8[:])
            nc.sync.dma_start(scores_sb[og * G : (og + 1) * G, :], stage8[:])

            if og < DC:
                # out_w bf16 casts, spread over the first groups (DVE slack)
                nc.vector.tensor_copy(out_w_bf[:, 2 * og, :], out_w_sb[:, 2 * og, :])
                nc.vector.tensor_copy(
                    out_w_bf[:, 2 * og + 1, :], out_w_sb[:, 2 * og + 1, :]
                )

            if og == NG // 2:
                # rows 0..31 complete since og 3: their softmax + mix runs
                # under og 5..7 (placed here so the ACT stream never blocks)
                _epilogue_softmax_mix(nc, 0, *sm_args)

        # keep the PE warm across the softmax wait before the h1 mix
        for k in range(16):
            wut = psum.tile([P, P], F32, tag="mm", name=f"wut_{k}")
            nc.tensor.matmul(wut[:], ident_bf[:], ident_bf[:], start=True, stop=True)

        _epilogue_softmax_mix(nc, 1, *sm_args)
        _final_project(nc, combT_bf, out_w_bf, ones_bf, outb_row_bf, out_sb,
                       psum, out_d)


_CACHE = {}


def build_nc():
    if "nc" in _CACHE:
        return _CACHE["nc"]
    nc = bacc.Bacc(
        "TRN2",
        target_bir_lowering=False,
        debug=False,
        num_devices=N_CORES,
    )
    with tile.TileContext(nc) as tc:
        _build_body(tc)
    nc.compile()
    _CACHE["nc"] = nc
    return nc


def kernel(**inputs):
    nc = build_nc()

    f = lambda k: np.ascontiguousarray(np.asarray(inputs[k], dtype=np.float32))
    output = f("output")
    context = f("context")
    shared = {
        "dec_w": f("dec_w"),
        "dec_b": f("dec_b").reshape(DEC, 1),
        "attn_w": f("attn_w"),
        "attn_b": f("attn_b").reshape(ATTN, 1),
        "query_w": f("query_w").reshape(DEC, 1),
        "out_w": f("out_w"),
        "out_b": f("out_b").reshape(DEC, 1),
    }
    in_maps = []
    for b in range(N_CORES):
        m = dict(shared)
        m["output"] = np.ascontiguousarray(output[b])
        m["context"] = np.ascontiguousarray(context[b])
        in_maps.append(m)

    res = bass_utils.run_bass_kernel_spmd(nc, in_maps, core_ids=list(range(N_CORES)))
    _CACHE["last_results"] = res
    out = np.stack([res.results[b]["out"] for b in range(N_CORES)])
    attn = np.stack([res.results[b]["attn"] for b in range(N_CORES)])
    return out, attn


# revision 19
# speedup vs baseline: 1.0301x; 1.0040x over previous
"""Bass/Tile Trainium2 kernel for additive (Bahdanau/'cat') attention.

Problem (per batch b):
  A[i,d]      = sum_a context[i,a] * attn_w[a,d] + attn_b[d]
  O[o,d]      = sum_e output[o,e]  * dec_w[e,d]  + dec_b[d]
  scores[o,i] = sum_d query_w[d] * tanh(A[i,d] + O[o,d])   (+query_b: softmax-invariant)
  attn        = softmax_i(scores)
  mix[o,a]    = sum_i attn[o,i] * context[i,a]
  out[o,d]    = tanh([mix | output] @ out_w + out_b)

Sharding: pure data-parallel over batch, B=8 -> one batch per NeuronCore,
weights broadcast, no collectives.

Per-core structure:
  * A^T [d,i] and O^T [d,o] kept with d on partitions so the broadcast add
    A^T + O^T[:,o] is a DVE tensor_scalar (per-partition scalar), in bf16.
  * tanh batched 8 o's per ACT instruction (free dim 4096); d-chunk-outer
    so the PE gets matmul work after every ACT chunk (keeps HAM warm).
  * q-reduction over d on the PE with zero-padded stationary operand:
    lhsT QZ[:,dc,j] is [128,8] holding query_w in column j -> all 32
    matmuls of a group accumulate into ONE [8,512] PSUM bank; one cheap
    8-row DVE copy + SBUF->SBUF DMA scatters rows into scores.
  * softmax/mix/out epilogue computed in row-halves (0:32 during groups
    4..7, 32:64 at the end) to shorten the serial tail.
"""

import numpy as np

import concourse.bass as bass
import concourse.tile as tile
import concourse.bass_utils as bass_utils
from concourse import bacc, mybir
from concourse.masks import make_identity

B, OUT_LEN, IN_LEN, DEC, ATTN = 8, 64, 512, 512, 512
P = 128
F32 = mybir.dt.float32
BF16 = mybir.dt.bfloat16
AF = mybir.ActivationFunctionType

G = 8                     # o's per tanh group
NG = OUT_LEN // G         # 8 groups
DC = DEC // P             # 4 d-chunks
AC = ATTN // P            # 4 a-chunks
IC = IN_LEN // P          # 4 i-chunks
EC = DEC // P             # 4 e-chunks (decoder feature)
CC = (ATTN + DEC) // P    # 8 combined chunks
H = OUT_LEN // 2          # row half

N_CORES = 8


def _epilogue_softmax_mix(nc, h, ident_bf, scores_sb, exp_sb, sums, recip,
                          attn_sb, attn_bf, attnT_bf, ctx_bf, combT_bf, psum,
                          attn_d):
    """softmax + attn^T + mix for rows h*32..h*32+31 (all-bf16 matmuls)."""
    r0 = h * H
    sl = slice(r0, r0 + H)
    nc.scalar.activation(exp_sb[sl, :], scores_sb[sl, :], AF.Exp, accum_out=sums[sl, :])
    nc.vector.reciprocal(recip[sl, :], sums[sl, :])
    nc.vector.tensor_scalar_mul(attn_sb[sl, :], exp_sb[sl, :], recip[sl, :])
    nc.sync.dma_start(attn_d[sl, :], attn_sb[sl, :])
    nc.vector.tensor_copy(attn_bf[sl, :], attn_sb[sl, :])

    for ic in range(IC):
        pt = psum.tile([P, H], BF16, tag="tp", name=f"pt_at_{h}_{ic}")
        nc.tensor.transpose(
            pt[:], attn_bf[sl, ic * P : (ic + 1) * P], ident_bf[sl, r0 : r0 + H]
        )
        nc.vector.tensor_copy(attnT_bf[:, ic, sl], pt[:])

    # mix^T -> combined chunks 0..3
    for ac in range(AC):
        pm = psum.tile([P, H], F32, tag="sm", name=f"pm_{h}_{ac}")
        for ic in range(IC):
            nc.tensor.matmul(
                pm[:],
                ctx_bf[:, ic, ac * P : (ac + 1) * P],
                attnT_bf[:, ic, sl],
                start=(ic == 0),
                stop=(ic == IC - 1),
            )
        nc.vector.tensor_copy(combT_bf[:, ac, sl], pm[:])


def _final_project(nc, combT_bf, out_w_bf, ones_bf, outb_row_bf, out_sb, psum,
                   out_d):
    """out = tanh(combined @ out_w + out_b) for all 64 rows at once:
    M=64 x N=512 matmuls, bias applied as a rank-1 (K=1) accumulation."""
    po = psum.tile([OUT_LEN, DEC], F32, tag="mm", name="po_final")
    for cc in range(CC):
        nc.tensor.matmul(
            po[:], combT_bf[:, cc, :], out_w_bf[:, cc, :],
            start=(cc == 0), stop=False,
        )
    nc.tensor.matmul(po[:], ones_bf[:], outb_row_bf[:], start=False, stop=True)
    nc.scalar.activation(out_sb[:], po[:], AF.Tanh)
    nc.sync.dma_start(out_d[:], out_sb[:])


def _build_body(tc):
    nc = tc.nc

    # ---- DRAM I/O (per-core shard shapes) ----
    output_d = nc.dram_tensor("output", [OUT_LEN, DEC], F32, kind="ExternalInput").ap()
    context_d = nc.dram_tensor("context", [IN_LEN, ATTN], F32, kind="ExternalInput").ap()
    dec_w_d = nc.dram_tensor("dec_w", [DEC, DEC], F32, kind="ExternalInput").ap()
    dec_b_d = nc.dram_tensor("dec_b", [DEC, 1], F32, kind="ExternalInput").ap()
    attn_w_d = nc.dram_tensor("attn_w", [ATTN, DEC], F32, kind="ExternalInput").ap()
    attn_b_d = nc.dram_tensor("attn_b", [ATTN, 1], F32, kind="ExternalInput").ap()
    query_w_d = nc.dram_tensor("query_w", [DEC, 1], F32, kind="ExternalInput").ap()
    out_w_d = nc.dram_tensor("out_w", [ATTN + DEC, DEC], F32, kind="ExternalInput").ap()
    out_b_d = nc.dram_tensor("out_b", [DEC, 1], F32, kind="ExternalInput").ap()
    out_d = nc.dram_tensor("out", [OUT_LEN, DEC], F32, kind="ExternalOutput").ap()
    attn_d = nc.dram_tensor("attn", [OUT_LEN, IN_LEN], F32, kind="ExternalOutput").ap()

    from contextlib import ExitStack

    with ExitStack() as ctx:
        const = ctx.enter_context(tc.tile_pool(name="const", bufs=1))
        statics = ctx.enter_context(tc.tile_pool(name="statics", bufs=1))
        epool = ctx.enter_context(tc.tile_pool(name="epool", bufs=2))
        fpool = ctx.enter_context(tc.tile_pool(name="fpool", bufs=2))
        spool = ctx.enter_context(tc.tile_pool(name="spool", bufs=2))
        psum = ctx.enter_context(tc.tile_pool(name="psum", bufs=2, space="PSUM"))

        # ---------------- constants / small inputs ----------------
        ident = const.tile([P, P], F32)
        make_identity(nc, ident)
        ident_bf = const.tile([P, P], BF16)
        nc.vector.tensor_copy(ident_bf[:], ident[:])

        # HAM warmup: ~4us of real matmul activity on dummy data flips the
        # PE clock gate to 8/8 (2.4 GHz) before the real matmuls arrive.
        # (PE-transpose-mode does not count as HAM activity.)
        wu = psum.tile([P, P], F32, tag="mm")
        for _ in range(16):
            nc.tensor.matmul(wu[:], ident_bf[:], ident_bf[:], start=True, stop=True)

        attn_bias = const.tile([P, DC], F32)
        dec_bias = const.tile([P, DC], F32)
        qw_f = const.tile([P, DC], F32)
        qw_bf = const.tile([P, DC], BF16)
        for tile_, dram_ in ((attn_bias, attn_b_d), (dec_bias, dec_b_d),
                             (qw_f, query_w_d)):
            nc.scalar.dma_start(
                tile_[:], dram_.rearrange("(dc p) one -> p dc one", p=P)
            )
        nc.vector.tensor_copy(qw_bf[:], qw_f[:])

        ones_bf = const.tile([1, OUT_LEN], BF16)
        nc.vector.memset(ones_bf[:], 1.0)
        outb_row_f = const.tile([1, DEC], F32)
        nc.scalar.dma_start(outb_row_f[:], out_b_d.rearrange("d one -> one d"))
        outb_row_bf = const.tile([1, DEC], BF16)
        nc.vector.tensor_copy(outb_row_bf[:], outb_row_f[:])


        # ---------------- big input DMAs (split for queue parallelism) ----
        ctx_sb = statics.tile([P, IC, ATTN], F32)      # [i%, ic, a]
        attn_w_sb = statics.tile([P, AC, DEC], F32)    # [a%, ac, d]
        dec_w_sb = statics.tile([P, EC, DEC], F32)     # [e%, ec, d]
        output_sb = statics.tile([OUT_LEN, DEC], F32)  # [o, e]
        out_w_sb = statics.tile([P, CC, DEC], F32)     # [c%, cc, d]
        ctx_bf = statics.tile([P, IC, ATTN], BF16)
        attn_w_bf = statics.tile([P, AC, DEC], BF16)
        dec_w_bf = statics.tile([P, EC, DEC], BF16)
        out_w_bf = statics.tile([P, CC, DEC], BF16)
        output_bf = statics.tile([OUT_LEN, DEC], BF16)
        for ic in range(IC):
            nc.sync.dma_start(ctx_sb[:, ic, :], context_d[ic * P : (ic + 1) * P, :])
        nc.scalar.dma_start(output_sb[:], output_d[:])
        for ac in range(AC):
            nc.scalar.dma_start(attn_w_sb[:, ac, :], attn_w_d[ac * P : (ac + 1) * P, :])
        for ec in range(EC):
            nc.sync.dma_start(dec_w_sb[:, ec, :], dec_w_d[ec * P : (ec + 1) * P, :])
        # bridge matmuls: keep the PE HAM-busy while DMAs land (paced by deps)
        for ic in range(IC):
            wub = psum.tile([P, ATTN], F32, tag="mm", name=f"wub_{ic}")
            nc.tensor.matmul(wub[:], ident[:], ctx_sb[:, ic, :], start=True, stop=True)
        for ic in range(IC):
            nc.vector.tensor_copy(ctx_bf[:, ic, :], ctx_sb[:, ic, :])
        nc.vector.tensor_copy(output_bf[:], output_sb[:])
        for ac in range(AC):
            nc.vector.tensor_copy(attn_w_bf[:, ac, :], attn_w_sb[:, ac, :])
        for ec in range(EC):
            nc.vector.tensor_copy(dec_w_bf[:, ec, :], dec_w_sb[:, ec, :])

        # ---------------- transposes: context^T (bf16), output^T ----------
        ctxT_bf = statics.tile([P, AC, IN_LEN], BF16)  # [a%, ac, i]
        for ic in range(IC):
            for ac in range(AC):
                pt = psum.tile([P, P], BF16, tag="tp", name=f"pt_c_{ic}_{ac}")
                nc.tensor.transpose(pt[:], ctx_bf[:, ic, ac * P : (ac + 1) * P], ident_bf[:])
                nc.vector.tensor_copy(ctxT_bf[:, ac, ic * P : (ic + 1) * P], pt[:])

        # combined^T [c%, cc, o]: chunks 0..3 = mix^T (later), 4..7 = output^T
        combT_bf = statics.tile([P, CC, OUT_LEN], BF16)
        for ec in range(EC):
            pt = psum.tile([P, OUT_LEN], BF16, tag="tp", name=f"pt_ot_{ec}")
            nc.tensor.transpose(
                pt[:], output_bf[0:OUT_LEN, ec * P : (ec + 1) * P],
                ident_bf[0:OUT_LEN, 0:OUT_LEN],
            )
            nc.vector.tensor_copy(combT_bf[:, EC + ec, :], pt[:])

        # ---------------- A^T then O^T ----------------
        ATb = statics.tile([P, DC, IN_LEN], BF16)      # [d%, dc, i]
        for dc in range(DC):
            pa = psum.tile([P, IN_LEN], F32, tag="mm", name=f"pa_{dc}")
            for ac in range(AC):
                nc.tensor.matmul(
                    pa[:],
                    attn_w_bf[:, ac, dc * P : (dc + 1) * P],
                    ctxT_bf[:, ac, :],
                    start=(ac == 0),
                    stop=(ac == AC - 1),
                )
            nc.vector.tensor_scalar_add(ATb[:, dc, :], pa[:], attn_bias[:, dc : dc + 1])

        OTb = statics.tile([P, DC, OUT_LEN], F32)      # [d%, dc, o]
        for dc in range(DC):
            po = psum.tile([P, OUT_LEN], F32, tag="sm", name=f"po_{dc}")
            for ec in range(EC):
                nc.tensor.matmul(
                    po[:],
                    dec_w_bf[:, ec, dc * P : (dc + 1) * P],
                    combT_bf[:, EC + ec, :],
                    start=(ec == 0),
                    stop=(ec == EC - 1),
                )
            nc.vector.tensor_scalar_add(OTb[:, dc, :], po[:], dec_bias[:, dc : dc + 1])

        # out_w lands during the main loop (needed first by epilogue half 0)
        for cc in range(CC):
            nc.sync.dma_start(out_w_sb[:, cc, :], out_w_d[cc * P : (cc + 1) * P, :])

        # zero-padded stationary operands: QZ[:, dc, j] is [128, G] with
        # query_w[dc] in column j, zeros elsewhere -> matmul j deposits
        # scores for o_j into PSUM row j, rows != j accumulate zeros.
        QZ = const.tile([P, DC, G, G], BF16)
        nc.vector.memset(QZ[:], 0.0)
        for dc in range(DC):
            for j in range(G):
                nc.vector.tensor_copy(QZ[:, dc, j, j : j + 1], qw_bf[:, dc : dc + 1])

        # ---------------- main loop: tanh + q-reduction ----------------
        scores_sb = statics.tile([OUT_LEN, IN_LEN], F32)
        exp_sb = statics.tile([OUT_LEN, IN_LEN], F32)
        sums = statics.tile([OUT_LEN, 1], F32)
        recip = statics.tile([OUT_LEN, 1], F32)
        attn_sb = statics.tile([OUT_LEN, IN_LEN], F32)
        attn_bf = statics.tile([OUT_LEN, IN_LEN], BF16)
        attnT_bf = statics.tile([P, IC, OUT_LEN], BF16)
        out_sb = statics.tile([OUT_LEN, DEC], F32)

        sm_args = (ident_bf, scores_sb, exp_sb, sums, recip, attn_sb, attn_bf,
                   attnT_bf, ctx_bf, combT_bf, psum, attn_d)

        for og in range(NG):
            ps8 = psum.tile([G, IN_LEN], F32, tag="sc", name=f"ps8_{og}")
            for dp in range(DC // 2):
                # two d-chunks per ACT instruction to amortize ACT overhead
                E = epool.tile([P, 2, G, IN_LEN], BF16, tag="E", name=f"E_{og}_{dp}")
                for c in range(2):
                    dc = 2 * dp + c
                    for j in range(G):
                        o = og * G + j
                        nc.vector.tensor_scalar_add(
                            E[:, c, j, :], ATb[:, dc, :], OTb[:, dc, o : o + 1]
                        )
                Fc = fpool.tile([P, 2, G, IN_LEN], BF16, tag="F", name=f"F_{og}_{dp}")
                nc.scalar.activation(Fc[:], E[:], AF.Tanh)
                for c in range(2):
                    dc = 2 * dp + c
                    for j in range(G):
                        nc.tensor.matmul(
                            ps8[:],
                            QZ[:, dc, j],
                            Fc[:, c, j],
                            start=(dc == 0 and j == 0),
                            stop=(dc == DC - 1 and j == G - 1),
                        )
            stage8 = spool.tile([G, IN_LEN], F32, tag="st", name=f"stage8_{og}")
            nc.vector.tensor_copy(stage8[:], ps8[:])
            nc.sync.dma_start(scores_sb[og * G : (og + 1) * G, :], stage8[:])

            if og < DC:
                # out_w bf16 casts, spread over the first groups (DVE slack)
                nc.vector.tensor_copy(out_w_bf[:, 2 * og, :], out_w_sb[:, 2 * og, :])
                nc.vector.tensor_copy(
                    out_w_bf[:, 2 * og + 1, :], out_w_sb[:, 2 * og + 1, :]
                )

            if og == NG // 2:
                # rows 0..31 complete since og 3: their softmax + mix runs
                # under og 5..7 (placed here so the ACT stream never blocks)
                _epilogue_softmax_mix(nc, 0, *sm_args)

        # keep the PE warm across the softmax wait before the h1 mix
        for k in range(16):
            wut = psum.tile([P, P], F32, tag="mm", name=f"wut_{k}")
            nc.tensor.matmul(wut[:], ident_bf[:], ident_bf[:], start=True, stop=True)

        _epilogue_softmax_mix(nc, 1, *sm_args)
        _final_project(nc, combT_bf, out_w_bf, ones_bf, outb_row_bf, out_sb,
                       psum, out_d)


_CACHE = {}


def build_nc():
    if "nc" in _CACHE:
        return _CACHE["nc"]
    nc = bacc.Bacc(
        "TRN2",
        target_bir_lowering=False,
        debug=False,
        num_devices=N_CORES,
    )
    with tile.TileContext(nc) as tc:
        _build_body(tc)
    nc.compile()
    _CACHE["nc"] = nc
    return nc


def kernel(**inputs):
    nc = build_nc()

    f = lambda k: np.ascontiguousarray(np.asarray(inputs[k], dtype=np.float32))
    output = f("output")
    context = f("context")
    shared = {
        "dec_w": f("dec_w"),
        "dec_b": f("dec_b").reshape(DEC, 1),
        "attn_w": f("attn_w"),
        "attn_b": f("attn_b").reshape(ATTN, 1),
        "query_w": f("query_w").reshape(DEC, 1),
        "out_w": f("out_w"),
        "out_b": f("out_b").reshape(DEC, 1),
    }
    in_maps = []
    for b in range(N_CORES):
        m = dict(shared)
        m["output"] = np.ascontiguousarray(output[b])
        m["context"] = np.ascontiguousarray(context[b])
        in_maps.append(m)

    res = bass_utils.run_bass_kernel_spmd(nc, in_maps, core_ids=list(range(N_CORES)))
    _CACHE["last_results"] = res
    out = np.stack([res.results[b]["out"] for b in range(N_CORES)])
    attn = np.stack([res.results[b]["attn"] for b in range(N_CORES)])
    return out, attn
